# revision 17
# baseline (speedup 1.0000x reference)
"""DeepSeek MoE layer on 8 Trainium2 NeuronCores — sparse routed experts.

Data-parallel over tokens (1024/core) with HOST-side load balancing: the
router is evaluated on host only to permute tokens across cores so every
core sees a near-uniform per-expert load, and to size the static per-expert
capacities (multiples of 128).  All real routing (logits, top-2, gates,
ranks, token lists) is recomputed ON DEVICE; host planning only picks
static shapes and the core assignment.

Per core:
  - fp32 router -> top-2 masks/gates (as dense baseline).
  - prefix-sum ranks via triangular matmuls; one-hot S tiles; fp32r
    extraction matmuls produce each expert's token-id row + gate row.
  - token ids -> int16 wrapped idx lists -> gpsimd.dma_gather (transpose)
    pulls each expert's tokens from the HBM fp8 hi/lo pair image of x.
  - sparse GELU experts in fp8 DoubleRow (hi/lo triple as baseline),
    phase-2 emitted in token-major orientation, scaled, and
    dma_scatter_add-ed (fp32, exact) into a DRAM y tile whose rows were
    initialised by the (transposed) shared-expert output.
  - shared SwiGLU experts keep the dense baseline path.

Expert matmuls run in fp8e4m3 DoubleRow with the hi+lo "triple" trick
(hi*hi + lo*hi + hi*lo) at 0.75 cyc per 128-contraction row — see the
dense baseline for the numerics rationale.
"""

import math

import numpy as np
import ml_dtypes

import concourse.bass as bass
import concourse.mybir as mybir
from concourse.tile import TileContext
from concourse.bass_utils import run_bass_kernel_spmd
from concourse import library_config

# ---------------------------------------------------------------------------
D = 1024          # d_model
HS = 2048         # shared expert hidden
HR = 1024         # routed expert hidden
E = 8             # routed experts
NS = 2            # shared experts
TOPK = 2
B, T = 4, 2048
N_CORES = 8
TOK = (B * T) // N_CORES      # tokens per core
P = 128
NKD = D // P                  # 8 k-subtiles over d_model
KPD = NKD // 2                # 4 k-pairs over d_model
MS = HS // P                  # 16 m-tiles shared hidden
MR = HR // P                  # 8 m-tiles routed hidden
NV = 512                      # moving-dim tile for shared phases
NN = TOK // NV
GKH = NS * MS                 # shared h k-subtiles (32)
W2SEG = 8192                  # shared w2 cols per m2 block
CAPMAX = 512                  # max per-expert capacity chunk held in PSUM

F32 = mybir.dt.float32
F32R = mybir.dt.float32r
BF16 = mybir.dt.bfloat16
F8 = mybir.dt.float8e4
I16 = mybir.dt.int16
DR = mybir.MatmulPerfMode.DoubleRow
F8NP = ml_dtypes.float8_e4m3

SW = 128.0        # w1/w2 quant scale
SW3 = 16.0        # shared w3 quant scale (folds the 1/8 shared combine)
SG = 32.0         # gate scale (folds the 1/4 routed combine at h scale 128)
SOUT = 1.0 / (SW * SW)
BIG = 65536.0


def _legalize_waits(nc):
    """Split multi-wait instructions into single-wait NOP prefixes (the
    walrus pass list used by the bass2jax compile path has no sync
    legalization pass; cayman 64B instructions carry one wait slot)."""
    n_split = 0
    for fn in nc.m.functions:
        for blk in fn.blocks:
            out = []
            changed = False
            for inst in blk.instructions:
                si = inst.sync_info
                waits = list(si.on_wait) if si is not None and si.on_wait else []
                if len(waits) > 1:
                    for w in waits[:-1]:
                        nop = mybir.InstNoOp(
                            name=nc.get_next_instruction_name(),
                            engine=inst.engine,
                            bass_nofuse=True,
                            sync_info=mybir.SyncInfo(on_wait=[w], on_update=[]),
                        )
                        nc.register_instruction(nop)
                        out.append(nop)
                    si.on_wait = [waits[-1]]
                    inst.sync_info = si
                    n_split += 1
                    changed = True
                out.append(inst)
            if changed:
                blk.instructions = out
    return n_split


def _build_nc(caps):
    """caps: tuple of E per-expert capacities (multiples of 128, may be 0)."""
    caps = tuple(int(c) for c in caps)
    assert all(0 <= c <= CAPMAX and c % P == 0 for c in caps), caps
    nc = bass.Bass()

    xT = nc.declare_dram_parameter("xT", [D, TOK], F32, isOutput=False)
    x8h_d = nc.declare_dram_parameter("x8h", [D, TOK], F8, isOutput=False)
    x8l_d = nc.declare_dram_parameter("x8l", [D, TOK], F8, isOutput=False)
    xpr_d = nc.declare_dram_parameter("xpr", [TOK + 16, 2 * D], F8, isOutput=False)
    tlog_d = nc.declare_dram_parameter("tlog", [TOK, E], F32, isOutput=False)
    wrx = nc.declare_dram_parameter("wrx", [D, E], F32, isOutput=False)
    ident = nc.declare_dram_parameter("ident", [P, P], F32, isOutput=False)
    iota_e = nc.declare_dram_parameter("iota", [P, E], F32, isOutput=False)
    u128 = nc.declare_dram_parameter("u128", [P, P], F32, isOutput=False)
    u8x = nc.declare_dram_parameter("u8x", [8, 8], F32, isOutput=False)
    tilesel = nc.declare_dram_parameter("tilesel", [P, 64], F32, isOutput=False)
    rowsel = nc.declare_dram_parameter("rowsel", [8, 1024], F32, isOutput=False)
    iotat = nc.declare_dram_parameter("iotat", [P, 64], F32R, isOutput=False)
    iotac = nc.declare_dram_parameter("iotac", [P, 4 * CAPMAX], F32, isOutput=False)
    w13s = nc.declare_dram_parameter("w13s", [P, NS * MS * 4 * NKD * P], F8, isOutput=False)
    w1r = nc.declare_dram_parameter("w1r", [P, E * MR * 2 * NKD * P], F8, isOutput=False)
    w2sp = nc.declare_dram_parameter("w2sp", [P, NKD * W2SEG], F8, isOutput=False)
    w2rt = nc.declare_dram_parameter("w2rt", [P, E * 2 * MR * D], F8, isOutput=False)
    yout = nc.declare_dram_parameter("yout", [TOK, D], F32, isOutput=True)

    AF = mybir.ActivationFunctionType
    ALU = mybir.AluOpType
    AX = mybir.AxisListType

    from contextlib import ExitStack
    with TileContext(nc) as tc:
        with ExitStack() as stk:
            def pool(name, bufs, space=None):
                kw = dict(space=space) if space else {}
                return stk.enter_context(tc.tile_pool(name=name, bufs=bufs, **kw))

            xpool = pool("xpool", 1)
            xfpool = pool("xfpool", 2)
            cpool = pool("cpool", 1)
            hpool = pool("hpool", 1)          # shared H (big)
            yapool = pool("yapool", 1)        # shared y accumulator
            w13pool = pool("w13pool", 2)
            w1rpool = pool("w1rpool", 4)
            w2spool = pool("w2spool", 2)
            w2rpool = pool("w2rpool", 2)
            rpool = pool("rpool", 1)          # router persistent tiles
            spool = pool("spool", 2)          # scratch vector tiles
            Spool = pool("Spool", 2)          # one-hot S tiles
            gxpool = pool("gxpool", 2)        # gathered x pairs
            ghpool = pool("ghpool", 1)        # routed H tiles
            gbpool = pool("gbpool", 2)
            ixpool = pool("ixpool", 1)        # wrapped idx int16 (per-e tags)
            apool = pool("apool", 2)
            hfpool = pool("hfpool", 2)
            ypool = pool("ypool", 2)          # shared transpose staging
            ytpool = pool("ytpool", 1)        # routed scatter staging
            dpool = pool("dpool", 4, "DRAM")
            pp_h1 = pool("pp_h1", 2, "PSUM")
            pp_h3 = pool("pp_h3", 2, "PSUM")
            pp_y = pool("pp_y", 2, "PSUM")
            pp_s = pool("pp_s", 2, "PSUM")

            nc.gpsimd.load_library(library_config.mlp)

            # ---------------- consts ----------------
            id_t = cpool.tile([P, P], F32)
            nc.sync.dma_start(out=id_t[:], in_=ident[:, :])
            io_t = cpool.tile([P, E], F32)
            nc.sync.dma_start(out=io_t[:], in_=iota_e[:, :])
            u_t = cpool.tile([P, P], F32)
            nc.sync.dma_start(out=u_t[:], in_=u128[:, :])
            u8_t = cpool.tile([8, 8], F32)
            nc.sync.dma_start(out=u8_t[:], in_=u8x[:, :])
            ts_t = cpool.tile([P, 64], F32)
            nc.sync.dma_start(out=ts_t[:], in_=tilesel[:, :])
            rs_t = cpool.tile([8, 1024], F32)
            nc.sync.dma_start(out=rs_t[:], in_=rowsel[:, :])
            it_t = cpool.tile([P, 64], F32R)
            nc.sync.dma_start(out=it_t[:], in_=iotat[:, :])
            ic_t = cpool.tile([P, 4 * CAPMAX], F32)
            nc.sync.dma_start(out=ic_t[:], in_=iotac[:, :])
            wr_t = cpool.tile([P, NKD * E], F32)     # [p, kd, e]
            nc.sync.dma_start(
                out=wr_t[:].rearrange("p (kd c) -> p kd c", c=E),
                in_=wrx.rearrange("(kd p) c -> p kd c", p=P),
            )
            ones32 = cpool.tile([1, P], F32)
            nc.vector.memset(ones32[:], 1.0)
            ones_r = cpool.tile([1, P], F32R)
            nc.vector.tensor_copy(ones_r[:], ones32[:])

            x8h = xpool.tile([P, NKD * TOK], F8)     # [p, kd, tok]
            x8l = xpool.tile([P, NKD * TOK], F8)
            nc.sync.dma_start(
                out=x8h[:].rearrange("p (kd t) -> p kd t", t=TOK),
                in_=x8h_d.rearrange("(kd p) t -> p kd t", p=P),
            )
            nc.sync.dma_start(
                out=x8l[:].rearrange("p (kd t) -> p kd t", t=TOK),
                in_=x8l_d.rearrange("(kd p) t -> p kd t", p=P),
            )
            xhv = x8h[:].rearrange("p (kd t) -> p kd t", t=TOK)
            xlv = x8l[:].rearrange("p (kd t) -> p kd t", t=TOK)

            # ---------------- router + masks + prefix ----------------
            NT = TOK // P                          # 8 token tiles
            gates_all = rpool.tile([P, NT * E], F32R)
            cm_all = rpool.tile([P, NT * E], F32)
            pref_all = rpool.tile([P, NT * E], F32)
            rkm_all = rpool.tile([P, NT * E], F32)

            tot_t = pp_y.tile([P, NV], F32, space="PSUM", tag="py")
            tot_ps = tot_t[0:8, 0:E]
            for tt in range(NT):
                xf = xfpool.tile([P, NKD * P], F32, tag="xf")
                nc.sync.dma_start(
                    out=xf[:].rearrange("p (kd t) -> p kd t", t=P),
                    in_=xT.rearrange("(kd p) t -> p kd t", p=P)[:, :, tt * P:(tt + 1) * P],
                )
                L_t = pp_h1.tile([P, NV], F32, space="PSUM", tag="ph1")
                L_ps = L_t[:, 0:E]
                for kd in range(NKD):
                    nc.tensor.matmul(
                        L_ps,
                        xf[:, kd * P:(kd + 1) * P],
                        wr_t[:, kd * E:(kd + 1) * E],
                        start=(kd == 0), stop=(kd == NKD - 1),
                    )
                tl_t = spool.tile([P, E], F32, tag="rt_tl")
                nc.sync.dma_start(out=tl_t[:], in_=tlog_d[tt * P:(tt + 1) * P, :])
                Lt = spool.tile([P, E], F32, tag="rt_L")
                nc.vector.tensor_copy(Lt[:], L_ps)
                Lx = spool.tile([P, E], F32, tag="rt_Lx")
                nc.vector.tensor_tensor(out=Lx[:], in0=Lt[:], in1=tl_t[:], op=ALU.add)
                St = spool.tile([P, E], F32, tag="rt_S")
                nc.scalar.activation(St[:], Lx[:], AF.Sigmoid)

                # top-1 (lowest index wins ties)
                m1 = spool.tile([P, 1], F32, tag="rt_m1")
                nc.vector.reduce_max(m1[:], Lx[:], axis=AX.X)
                eq1 = spool.tile([P, E], F32, tag="rt_eq1")
                nc.vector.tensor_scalar(eq1[:], Lx[:], m1[:, 0:1], None, op0=ALU.is_ge)
                pen1 = spool.tile([P, E], F32, tag="rt_pen1")
                nc.vector.tensor_scalar(pen1[:], eq1[:], -1e9, 1e9,
                                        op0=ALU.mult, op1=ALU.add)
                ix1 = spool.tile([P, E], F32, tag="rt_ix1")
                nc.vector.tensor_tensor(out=ix1[:], in0=io_t[:], in1=pen1[:], op=ALU.add)
                i1 = spool.tile([P, 1], F32, tag="rt_i1")
                nc.vector.tensor_reduce(i1[:], ix1[:], axis=AX.X, op=ALU.min)
                mask1 = spool.tile([P, E], F32, tag="rt_mask1")
                nc.vector.tensor_scalar(mask1[:], io_t[:], i1[:, 0:1], None,
                                        op0=ALU.is_equal)

                # top-2
                neg1 = spool.tile([P, E], F32, tag="rt_neg1")
                nc.vector.tensor_scalar(neg1[:], mask1[:], -1e30, None, op0=ALU.mult)
                L2 = spool.tile([P, E], F32, tag="rt_L2")
                nc.vector.tensor_tensor(out=L2[:], in0=Lx[:], in1=neg1[:], op=ALU.add)
                m2t = spool.tile([P, 1], F32, tag="rt_m2")
                nc.vector.reduce_max(m2t[:], L2[:], axis=AX.X)
                eq2 = spool.tile([P, E], F32, tag="rt_eq2")
                nc.vector.tensor_scalar(eq2[:], L2[:], m2t[:, 0:1], None, op0=ALU.is_ge)
                pen2 = spool.tile([P, E], F32, tag="rt_pen2")
                nc.vector.tensor_scalar(pen2[:], eq2[:], -1e9, 1e9,
                                        op0=ALU.mult, op1=ALU.add)
                ix2 = spool.tile([P, E], F32, tag="rt_ix2")
                nc.vector.tensor_tensor(out=ix2[:], in0=io_t[:], in1=pen2[:], op=ALU.add)
                i2 = spool.tile([P, 1], F32, tag="rt_i2")
                nc.vector.tensor_reduce(i2[:], ix2[:], axis=AX.X, op=ALU.min)
                mask2 = spool.tile([P, E], F32, tag="rt_mask2")
                nc.vector.tensor_scalar(mask2[:], io_t[:], i2[:, 0:1], None,
                                        op0=ALU.is_equal)

                mask = spool.tile([P, E], F32, tag="rt_mask")
                nc.vector.tensor_tensor(out=mask[:], in0=mask1[:], in1=mask2[:], op=ALU.add)
                sm = spool.tile([P, E], F32, tag="rt_sm")
                nc.vector.tensor_tensor(out=sm[:], in0=St[:], in1=mask[:], op=ALU.mult)
                den = spool.tile([P, 1], F32, tag="rt_den")
                nc.vector.reduce_sum(den[:], sm[:], axis=AX.X)
                den2 = spool.tile([P, 1], F32, tag="rt_den2")
                nc.vector.tensor_scalar(den2[:], den[:], 1e-9, None, op0=ALU.add)
                rec = spool.tile([P, 1], F32, tag="rt_rec")
                nc.vector.reciprocal(rec[:], den2[:])
                recs = spool.tile([P, 1], F32, tag="rt_recs")
                nc.vector.tensor_scalar(recs[:], rec[:], SG, None, op0=ALU.mult)
                esl = slice(tt * E, (tt + 1) * E)
                nc.vector.tensor_scalar(gates_all[:, esl], sm[:], recs[:, 0:1],
                                        None, op0=ALU.mult)
                nc.vector.tensor_copy(cm_all[:, esl], mask[:])

                # in-tile inclusive prefix over tokens (contraction over t')
                pf_t = pp_h1.tile([P, NV], F32, space="PSUM", tag="ph1")
                pf_ps = pf_t[:, 0:E]
                nc.tensor.matmul(pf_ps, u_t[:], mask[:], start=True, stop=True)
                nc.vector.tensor_copy(pref_all[:, esl], pf_ps)
                # tile totals accumulate into row tt of [8, E]
                nc.tensor.matmul(tot_ps, ts_t[:, tt * 8:(tt + 1) * 8], mask[:],
                                 start=(tt == 0), stop=(tt == NT - 1))

            tot_sb = spool.tile([8, E], F32, tag="tot")
            nc.vector.tensor_copy(tot_sb[:], tot_ps)
            off_t = pp_y.tile([P, NV], F32, space="PSUM", tag="py")
            off_ps = off_t[0:8, 0:E]
            nc.tensor.matmul(off_ps, u8_t[:], tot_sb[:], start=True, stop=True)
            off_sb = spool.tile([8, E], F32, tag="off")
            nc.vector.tensor_copy(off_sb[:], off_ps)

            for tt in range(NT):
                esl = slice(tt * E, (tt + 1) * E)
                ob_t = pp_h1.tile([P, NV], F32, space="PSUM", tag="ph1")
                ob_ps = ob_t[:, 0:E]
                nc.tensor.matmul(ob_ps, rs_t[:, tt * P:(tt + 1) * P], off_sb[:],
                                 start=True, stop=True)
                rk = spool.tile([P, E], F32, tag="rk")
                nc.vector.tensor_copy(rk[:], ob_ps)
                rk2 = spool.tile([P, E], F32, tag="rk2")
                nc.vector.tensor_tensor(out=rk2[:], in0=rk[:], in1=pref_all[:, esl],
                                        op=ALU.add)
                # rank = rk2 - 1 where member else -BIG:
                rk3 = spool.tile([P, E], F32, tag="rk3")
                nc.vector.tensor_scalar(rk3[:], rk2[:], BIG - 1.0, None, op0=ALU.add)
                rk4 = spool.tile([P, E], F32, tag="rk4")
                nc.vector.tensor_tensor(out=rk4[:], in0=rk3[:], in1=cm_all[:, esl],
                                        op=ALU.mult)
                nc.vector.tensor_scalar(rkm_all[:, esl], rk4[:], -BIG, None,
                                        op0=ALU.add)

            # ---------------- per-expert token lists + gathers ----------------
            idx16 = {}
            elist = [e for e in range(E) if caps[e] > 0]
            NJX = max(caps) // P
            grow_all = rpool.tile([8, CAPMAX], F32R)
            for e in elist:
                C = caps[e]
                ex_t = pp_y.tile([P, NV], F32, space="PSUM", tag="py")
                ex_ps = ex_t[0:9, 0:C]
                icoff = (C // P - 1) * CAPMAX
                for tt in range(NT):
                    esl = tt * E + e
                    stg = spool.tile([P, 9], F32R, tag="stg")
                    nc.vector.tensor_copy(stg[:, 0:8], it_t[:, tt * 8:(tt + 1) * 8])
                    nc.vector.tensor_copy(stg[:, 8:9], gates_all[:, esl:esl + 1])
                    S_t = Spool.tile([P, CAPMAX], F32R, tag="S")
                    nc.vector.tensor_scalar(S_t[:, :C], ic_t[:, icoff:icoff + C],
                                            rkm_all[:, esl:esl + 1], None,
                                            op0=ALU.is_equal)
                    nc.tensor.matmul(ex_ps, stg[:, 0:9], S_t[:, :C],
                                     start=(tt == 0), stop=(tt == NT - 1))
                gall = spool.tile([16, CAPMAX], F32, tag="gall")
                nc.vector.tensor_copy(gall[0:9, :C], ex_t[0:9, 0:C])
                # iotat carries (token_id - TOK): empty slots sum to 0, so
                # +TOK points them at the sink row / zero pad row.
                ixa = spool.tile([8, CAPMAX], F32, tag="ixa")
                nc.vector.tensor_scalar(ixa[:, :C], gall[0:8, :C], float(TOK),
                                        None, op0=ALU.add)
                ixi = spool.tile([8, CAPMAX], I16, tag="ixi")
                nc.vector.tensor_copy(ixi[:, :C], ixa[:, :C])
                nc.sync.dma_start(out=grow_all[e:e + 1, :C].bitcast(F32), in_=gall[8:9, :C])

                dsc = dpool.tile([8, C], I16, tag=f"dsc{e % 4}")
                nc.sync.dma_start(out=dsc[:, :], in_=ixi[:8, :C])
                ixw = ixpool.tile([P, C // 16], I16, tag=f"ix{e}")
                nc.sync.dma_start(
                    out=ixw[:],
                    in_=dsc[:, :].rearrange("r (g m) -> (r g) m", g=16),
                )
                idx16[e] = ixw

            # ---------------- shared experts (per-expert groups) ------------
            # H for ONE shared expert at a time (halves SBUF vs both).
            y_acc = yapool.tile([P, NKD * TOK], BF16)     # [p, m2, tok]

            def triple(ps, wh, wl, rh, rl, first, last, kpr):
                for kp in range(kpr):
                    ks = slice(2 * kp, 2 * kp + 2)
                    nc.tensor.matmul(ps, wh[:, ks], rh[:, ks],
                                     start=(first and kp == 0), stop=False,
                                     perf_mode=DR)
                    nc.tensor.matmul(ps, wh[:, ks], rl[:, ks],
                                     start=False, stop=False, perf_mode=DR)
                    nc.tensor.matmul(ps, wl[:, ks], rh[:, ks],
                                     start=False, stop=(last and kp == kpr - 1),
                                     perf_mode=DR)

            for se in range(NS):
                hh = hpool.tile([P, MS * TOK], F8, tag="hh")
                hl = hpool.tile([P, MS * TOK], F8, tag="hl")
                hhv = hh[:].rearrange("p (kh t) -> p kh t", t=TOK)
                hlv = hl[:].rearrange("p (kh t) -> p kh t", t=TOK)
                for m in range(MS):
                    w13 = w13pool.tile([P, 4 * NKD * P], F8, tag="w13")
                    off = (se * MS + m) * 4 * NKD * P
                    nc.sync.dma_start(out=w13[:], in_=w13s[:, off:off + 4 * NKD * P])
                    wv = w13[:].rearrange("p (v ks mm) -> p v ks mm", v=4, mm=P)
                    for n in range(NN):
                        nsl = slice(n * NV, (n + 1) * NV)
                        ps1 = pp_h1.tile([P, NV], F32, space="PSUM", tag="ph1")
                        triple(ps1[:], wv[:, 0], wv[:, 1],
                               xhv[:, :, nsl], xlv[:, :, nsl], True, True, KPD)
                        ps3 = pp_h3.tile([P, NV], F32, space="PSUM", tag="ph3")
                        triple(ps3[:], wv[:, 2], wv[:, 3],
                               xhv[:, :, nsl], xlv[:, :, nsl], True, True, KPD)
                        sil = apool.tile([P, NV], BF16, tag="sil")
                        nc.scalar.activation(sil[:], ps1[:], AF.Silu, scale=1.0 / SW)
                        hf = hfpool.tile([P, NV], BF16, tag="hf")
                        nc.vector.tensor_tensor(out=hf[:], in0=sil[:], in1=ps3[:],
                                                op=ALU.mult)
                        nc.scalar.copy(hhv[:, m, nsl], hf[:])
                        nc.vector.tensor_tensor(out=hlv[:, m, nsl], in0=hf[:],
                                                in1=hhv[:, m, nsl], op=ALU.subtract)

                # phase 2 for this shared expert: accumulate into y_acc
                for m2 in range(NKD):
                    w2t = w2spool.tile([P, W2SEG // 2], F8, tag="w2s")
                    off = m2 * W2SEG + se * (W2SEG // 2)
                    nc.sync.dma_start(out=w2t[:], in_=w2sp[:, off:off + W2SEG // 2])
                    w2v = w2t[:].rearrange("p (v ks mm) -> p v ks mm", v=2, mm=P)
                    for n in range(NN):
                        nsl = slice(n * NV, (n + 1) * NV)
                        py = pp_y.tile([P, NV], F32, space="PSUM", tag="py")
                        nmm = (MS // 2) * 3
                        i = 0
                        for kp in range(MS // 2):
                            kh = slice(2 * kp, 2 * kp + 2)
                            ks = slice(2 * kp, 2 * kp + 2)
                            nc.tensor.matmul(py[:], w2v[:, 0, ks, :],
                                             hhv[:, kh, nsl],
                                             start=(i == 0), stop=False,
                                             perf_mode=DR)
                            i += 1
                            nc.tensor.matmul(py[:], w2v[:, 0, ks, :],
                                             hlv[:, kh, nsl],
                                             start=False, stop=False,
                                             perf_mode=DR)
                            i += 1
                            nc.tensor.matmul(py[:], w2v[:, 1, ks, :],
                                             hhv[:, kh, nsl],
                                             start=False, stop=(i == nmm - 1),
                                             perf_mode=DR)
                            i += 1
                        ysl = y_acc[:, m2 * TOK + n * NV:m2 * TOK + n * NV + NV]
                        if se == 0:
                            nc.scalar.copy(ysl, py[:])
                        else:
                            nc.vector.tensor_tensor(out=ysl, in0=ysl, in1=py[:],
                                                    op=ALU.add)

            # ---------------- shared transpose -> y_dram rows ----------------
            id_bf = cpool.tile([P, P], BF16)
            nc.vector.tensor_copy(id_bf[:], id_t[:])
            y_dram = dpool.tile([TOK + P, D], F32, tag="ydram")
            for tt in range(NT):
                yts = ypool.tile([P, D], F32, tag="yts")
                for m2 in range(NKD):
                    tr_ps = pp_s.tile([P, P], BF16, space="PSUM", tag="ps_tr")
                    nc.tensor.transpose(
                        out=tr_ps[:],
                        in_=y_acc[:, m2 * TOK + tt * P:m2 * TOK + (tt + 1) * P],
                        identity=id_bf[:])
                    nc.scalar.mul(yts[:, m2 * P:(m2 + 1) * P], tr_ps[:], SOUT)
                nc.sync.dma_start(out=y_dram[tt * P:(tt + 1) * P, :], in_=yts[:])

            # ---------------- routed experts (sparse) ----------------
            for e in elist:
                C = caps[e]
                NJ = C // P
                g_t = gxpool.tile([P, 16 * C], F8, tag="xg")
                nc.gpsimd.dma_gather(
                    g_t[:].rearrange("p (a b) -> p a b", a=16),
                    xpr_d[:, :], idx16[e][:], C, C, 2 * D, transpose=True)
                xv = g_t[:].rearrange("p (c i v) -> p c i v", c=NKD, v=2)
                xgh = xv[:, :, :, 0]
                xgl = xv[:, :, :, 1]

                gb_ps = pp_h3.tile([P, CAPMAX], F32, space="PSUM", tag="ph3")
                gr0 = gbpool.tile([1, CAPMAX], F32R, tag="gr0")
                nc.sync.dma_start(out=gr0[0:1, :C], in_=grow_all[e:e + 1, :C])
                gr_v = (gr0[0:1, :C]
                        .rearrange("o (g m) -> o g m", g=16)
                        .rearrange("o g m -> o m g"))
                gb_v = gb_ps[:, :C].rearrange("p (m g) -> p m g", g=16)
                nc.tensor.matmul(gb_v, ones_r[0:1, :], gr_v,
                                 start=True, stop=True)
                gb = gbpool.tile([P, CAPMAX], F32, tag="gb")
                nc.vector.tensor_copy(gb[:, :C], gb_ps[:, :C])

                # phase 1: H = gelu(x W1) * gate   [h-part, C]
                ghh = ghpool.tile([P, MR * CAPMAX], F8, tag="ghh")
                ghl = ghpool.tile([P, MR * CAPMAX], F8, tag="ghl")
                ghhv = ghh[:].rearrange("p (kh c) -> p kh c", c=CAPMAX)
                ghlv = ghl[:].rearrange("p (kh c) -> p kh c", c=CAPMAX)
                for m in range(MR):
                    w1t = w1rpool.tile([P, 2 * NKD * P], F8, tag="w1r")
                    off = (e * MR + m) * 2 * NKD * P
                    nc.sync.dma_start(out=w1t[:], in_=w1r[:, off:off + 2 * NKD * P])
                    wv = w1t[:].rearrange("p (v ks mm) -> p v ks mm", v=2, mm=P)
                    ps1 = pp_h1.tile([P, CAPMAX], F32, space="PSUM", tag="ph1")
                    triple(ps1[:, :C], wv[:, 0], wv[:, 1],
                           xgh[:, :, :C], xgl[:, :, :C], True, True, KPD)
                    gel = apool.tile([P, CAPMAX], BF16, tag="gel")
                    nc.scalar.activation(gel[:, :C], ps1[:, :C], AF.Gelu,
                                         scale=1.0 / SW)
                    hf = hfpool.tile([P, CAPMAX], BF16, tag="ghf")
                    nc.vector.tensor_tensor(out=hf[:, :C], in0=gel[:, :C],
                                            in1=gb[:, :C], op=ALU.mult)
                    nc.scalar.copy(ghhv[:, m, :C], hf[:, :C])
                    nc.vector.tensor_tensor(out=ghlv[:, m, :C], in0=hf[:, :C],
                                            in1=ghhv[:, m, :C], op=ALU.subtract)

                # phase 2 flipped: Y^T [slot, d] ; contraction over h
                yt = ytpool.tile([P, NJX * D], F32, tag="yt")
                ytv = yt[:, :NJ * D].rearrange("p (j d) -> p j d", d=D)
                for half in range(2):
                    dsl = slice(half * (D // 2), (half + 1) * (D // 2))
                    pys = []
                    for j in range(NJ):
                        if j < 2:
                            pyj = pp_y.tile([P, D // 2], F32, space="PSUM",
                                            tag="py", name=f"pyj{j}")
                        else:
                            pyj = pp_h1.tile([P, D // 2], F32, space="PSUM",
                                             tag="ph1", name=f"pyj{j}")
                        pys.append(pyj)
                    for kp in range(MR // 2):
                        w2c = w2rpool.tile([P, 2 * 2 * (D // 2)], F8, tag="w2r")
                        cw = 2 * 2 * (D // 2)
                        coff = ((e * (MR // 2) + kp) * 2 + half) * cw
                        nc.sync.dma_start(out=w2c[:], in_=w2rt[:, coff:coff + cw])
                        wc = w2c[:].rearrange("p (v ks d) -> p v ks d", v=2, ks=2)
                        for j in range(NJ):
                            jsl = slice(j * P, (j + 1) * P)
                            hsl = slice(2 * kp, 2 * kp + 2)
                            first = (kp == 0)
                            last = (kp == MR // 2 - 1)
                            nc.tensor.matmul(pys[j][:], ghhv[:, hsl, jsl],
                                             wc[:, 0], start=first, stop=False,
                                             perf_mode=DR)
                            nc.tensor.matmul(pys[j][:], ghlv[:, hsl, jsl],
                                             wc[:, 0], start=False, stop=False,
                                             perf_mode=DR)
                            nc.tensor.matmul(pys[j][:], ghhv[:, hsl, jsl],
                                             wc[:, 1], start=False, stop=last,
                                             perf_mode=DR)
                    for j in range(NJ):
                        nc.scalar.mul(ytv[:, j, dsl], pys[j][:], SOUT)

                # scatter-add into y rows (exact fp32); pads add zeros to row 0
                nc.gpsimd.dma_scatter_add(
                    y_dram[:, :], ytv[:, :, :], idx16[e][:], C, C, D)

            # ---------------- final copy ----------------
            nc.sync.dma_start(out=yout[:, :], in_=y_dram[0:TOK, :])

    _legalize_waits(nc)
    mybir.codegen_inst_isa_subclasses(nc)
    return nc


# ---------------------------------------------------------------------------
_CACHE = {}


def _hilo(w, scale):
    v = np.asarray(w, np.float32) * scale
    hi = np.clip(v, -240.0, 240.0).astype(F8NP)
    lo = (v - hi.astype(np.float32)).astype(F8NP)
    return hi, lo


def _pack_in(a):
    E_, H, Dd = a.shape
    M = H // P
    KS = Dd // P
    a = a.reshape(E_, M, P, KS, P).transpose(4, 0, 1, 3, 2)
    return np.ascontiguousarray(a.reshape(P, E_, M, KS * P))


def _pack_out(a):
    E_, Dd, H = a.shape
    M2 = Dd // P
    KS = H // P
    return a.reshape(E_, M2, P, KS, P).transpose(4, 0, 1, 3, 2)


def _prep_weights(W_router, router_bias, s_w1, s_w3, s_w2, r_w1, r_w2):
    key = tuple(id(a) for a in (W_router, router_bias, s_w1, s_w3, s_w2, r_w1, r_w2))
    hit = _CACHE.get("wkey")
    if hit is not None and hit[0] == key:
        return hit[1]
    assert np.all(np.asarray(router_bias) == 0.0), "kernel assumes zero router bias"
    c = np.ascontiguousarray
    f = np.float32

    w1h, w1l = _hilo(s_w1, SW)
    w3h, w3l = _hilo(s_w3, SW3)
    parts = [_pack_in(a) for a in (w1h, w1l, w3h, w3l)]
    w13 = np.stack(parts, axis=3)
    w13 = c(w13.reshape(P, -1))

    r1h, r1l = _hilo(r_w1, SW)
    w1rp = np.stack([_pack_in(r1h), _pack_in(r1l)], axis=3)
    w1rp = c(w1rp.reshape(P, -1))

    # shared w2 blocks per m2 (dense path): [p, e, hi|lo, ks, mm]
    s2h, s2l = _hilo(s_w2, SW)
    w2s = np.stack([_pack_out(s2h), _pack_out(s2l)], axis=2)  # [p, e, 2, m2, ks, mm]
    segs = [w2s[:, :, :, m2].reshape(P, -1) for m2 in range(NKD)]
    w2spk = c(np.concatenate(segs, axis=1))

    # routed w2 transposed for flipped phase 2: contraction over h.
    # r_w2: [E, D, HR] -> w2T [E, HR, D] -> [p=h%128, e, v, ks=h//128, d]
    r2h, r2l = _hilo(r_w2, SW)
    def packT(a):
        aT = np.ascontiguousarray(np.transpose(a, (0, 2, 1)))   # [E, HR, D]
        return aT.reshape(E, MR, P, D).transpose(2, 0, 1, 3)    # [p, e, ks, d]
    w2rt = np.stack([packT(r2h), packT(r2l)], axis=2)           # [p, e, v, ks, d]
    # -> [p, e, kp, half, v, ks2, d512] contiguous per (e, kp, half) chunk
    w2rt = w2rt.reshape(P, E, 2, MR // 2, 2, 2, D // 2)         # [p,e,v,kp,ks2,half,d]
    w2rt = w2rt.transpose(0, 1, 3, 5, 2, 4, 6)                  # [p,e,kp,half,v,ks2,d]
    w2rt = c(w2rt.reshape(P, -1))

    wrTf = c(np.asarray(W_router, f).T)             # [2D, E]
    iotac = np.full((P, 4 * CAPMAX), -2.0, f)
    for k in range(4):
        C = (k + 1) * P
        j = np.arange(C)
        vals = (j % (C // 16)) * 16 + j // (C // 16)
        iotac[:, k * CAPMAX:k * CAPMAX + C] = vals[None, :].astype(f)
    u128 = np.triu(np.ones((P, P), f))              # u[t', t] = t' <= t
    u8x = np.triu(np.ones((8, 8), f), 1)            # exclusive
    tilesel = np.zeros((P, 8, 8), f)
    for i in range(8):
        tilesel[:, i, i] = 1.0
    rowsel = np.zeros((8, 8, P), f)
    for i in range(8):
        rowsel[i, i, :] = 1.0
    iotat = np.zeros((P, 8, 8), f)
    for tt in range(8):
        iotat[:, tt, :] = (tt * P + np.arange(P, dtype=f) - TOK)[:, None]

    prep = dict(
        w13s=w13, w1r=w1rp, w2sp=w2spk, w2rt=w2rt,
        wrx=c(wrTf[:D, :]),
        wrt_t=c(wrTf[D:, :]),
        iota=c(np.broadcast_to(np.arange(E, dtype=f), (P, E))),
        ident=np.eye(P, dtype=f),
        u128=c(u128), u8x=c(u8x),
        tilesel=c(tilesel.reshape(P, 64)),
        rowsel=c(rowsel.reshape(8, 1024)),
        iotat=c(iotat.reshape(P, 64)),
        iotac=c(iotac),
    )
    _CACHE["wkey"] = (key, prep)
    return prep


def _plan_routing(x, t_emb, W_router, router_bias):
    """Host-side planning: per-core token permutation + static capacities.

    Only sizes/placement come from here; the device recomputes routing."""
    f = np.float32
    N = B * T
    xf = np.asarray(x, f).reshape(N, D)
    W = np.asarray(W_router, f)
    tlogB = np.asarray(t_emb, f) @ W[:, D:].T          # [B, E]
    tlogN = np.repeat(tlogB, T, axis=0)                # [N, E]
    logits = xf @ W[:, :D].T + tlogN
    s = 1.0 / (1.0 + np.exp(-logits))
    sel = s + np.asarray(router_bias, f)
    top1 = np.argmax(sel, axis=1)
    sel2 = sel.copy()
    sel2[np.arange(N), top1] = -np.inf
    top2 = np.argmax(sel2, axis=1)

    # greedy balancing: assign each token to the core with the lightest
    # combined load on its two experts.
    load = np.zeros((N_CORES, E), np.int32)
    fill = np.zeros(N_CORES, np.int32)
    core_of = np.empty(N, np.int32)
    order = np.argsort(top1 * E + top2, kind="stable")
    for t in order:
        e1, e2 = top1[t], top2[t]
        best, bestv = -1, None
        for cix in range(N_CORES):
            if fill[cix] >= TOK:
                continue
            v = (int(load[cix][e1]) + int(load[cix][e2]), int(fill[cix]))
            if bestv is None or v < bestv:
                best, bestv = cix, v
        core_of[t] = best
        fill[best] += 1
        load[best][e1] += 1
        load[best][e2] += 1

    perm = np.argsort(core_of, kind="stable")          # tokens grouped by core
    caps = []
    for e in range(E):
        worst = int(load[:, e].max())
        caps.append(min(int(math.ceil((worst + 64) / P)) * P, CAPMAX))
    return perm, tuple(caps), tlogN


def kernel(x, t_emb, W_router, router_bias, s_w1, s_w3, s_w2, r_w1, r_w2):
    x = np.asarray(x, np.float32)
    pw = _prep_weights(W_router, router_bias, s_w1, s_w3, s_w2, r_w1, r_w2)
    perm, caps, tlogN = _plan_routing(x, t_emb, W_router, router_bias)
    _CACHE["caps"] = caps

    if _CACHE.get("nc_caps") != caps:
        _CACHE["nc"] = _build_nc(caps)
        _CACHE["nc_caps"] = caps
    nc = _CACHE["nc"]

    N = B * T
    x_rows = x.reshape(N, D)[perm]                     # permuted token rows
    xT_full = np.ascontiguousarray(x_rows.T)           # [D, N]
    xh_full, xl_full = _hilo(xT_full, 1.0)
    xh_rows, xl_rows = _hilo(x_rows, 1.0)
    xpr_full = np.stack([xh_rows, xl_rows], axis=-1).reshape(N, 2 * D)
    tlog_perm = np.ascontiguousarray(tlogN[perm])

    in_maps = []
    for cix in range(N_CORES):
        sl = slice(cix * TOK, (cix + 1) * TOK)
        in_maps.append(dict(
            xT=np.ascontiguousarray(xT_full[:, sl]),
            x8h=np.ascontiguousarray(xh_full[:, sl]),
            x8l=np.ascontiguousarray(xl_full[:, sl]),
            xpr=np.ascontiguousarray(np.concatenate(
                [xpr_full[sl], np.zeros((16, 2 * D), F8NP)], axis=0)),
            tlog=np.ascontiguousarray(tlog_perm[sl]),
            wrx=pw["wrx"], ident=pw["ident"], iota=pw["iota"],
            u128=pw["u128"], u8x=pw["u8x"], tilesel=pw["tilesel"],
            rowsel=pw["rowsel"], iotat=pw["iotat"], iotac=pw["iotac"],
            w13s=pw["w13s"], w1r=pw["w1r"], w2sp=pw["w2sp"], w2rt=pw["w2rt"],
        ))

    res = run_bass_kernel_spmd(nc, in_maps, list(range(N_CORES)))

    out_perm = np.empty((N, D), dtype=np.float32)
    for cix in range(N_CORES):
        out_perm[cix * TOK:(cix + 1) * TOK] = res.results[cix]["yout"]
    out = np.empty((N, D), dtype=np.float32)
    out[perm] = out_perm
    return out.reshape(B, T, D)


# revision 18
# speedup vs baseline: 1.0025x; 1.0025x over previous
"""DeepSeek MoE layer on 8 Trainium2 NeuronCores — sparse routed experts.

Data-parallel over tokens (1024/core) with HOST-side load balancing: the
router is evaluated on host only to permute tokens across cores so every
core sees a near-uniform per-expert load, and to size the static per-expert
capacities (multiples of 128).  All real routing (logits, top-2, gates,
ranks, token lists) is recomputed ON DEVICE; host planning only picks
static shapes and the core assignment.

Per core:
  - fp32 router -> top-2 masks/gates (as dense baseline).
  - prefix-sum ranks via triangular matmuls; one-hot S tiles; fp32r
    extraction matmuls produce each expert's token-id row + gate row.
  - token ids -> int16 wrapped idx lists -> gpsimd.dma_gather (transpose)
    pulls each expert's tokens from the HBM fp8 hi/lo pair image of x.
  - sparse GELU experts in fp8 DoubleRow (hi/lo triple as baseline),
    phase-2 emitted in token-major orientation, scaled, and
    dma_scatter_add-ed (fp32, exact) into a DRAM y tile whose rows were
    initialised by the (transposed) shared-expert output.
  - shared SwiGLU experts keep the dense baseline path.

Expert matmuls run in fp8e4m3 DoubleRow with the hi+lo "triple" trick
(hi*hi + lo*hi + hi*lo) at 0.75 cyc per 128-contraction row — see the
dense baseline for the numerics rationale.
"""

import math

import numpy as np
import ml_dtypes

import concourse.bass as bass
import concourse.mybir as mybir
from concourse.tile import TileContext
from concourse.bass_utils import run_bass_kernel_spmd
from concourse import library_config

# ---------------------------------------------------------------------------
D = 1024          # d_model
HS = 2048         # shared expert hidden
HR = 1024         # routed expert hidden
E = 8             # routed experts
NS = 2            # shared experts
TOPK = 2
B, T = 4, 2048
N_CORES = 8
TOK = (B * T) // N_CORES      # tokens per core
P = 128
NKD = D // P                  # 8 k-subtiles over d_model
KPD = NKD // 2                # 4 k-pairs over d_model
MS = HS // P                  # 16 m-tiles shared hidden
MR = HR // P                  # 8 m-tiles routed hidden
NV = 512                      # moving-dim tile for shared phases
NN = TOK // NV
GKH = NS * MS                 # shared h k-subtiles (32)
W2SEG = 8192                  # shared w2 cols per m2 block
CAPMAX = 512                  # max per-expert capacity chunk held in PSUM

F32 = mybir.dt.float32
F32R = mybir.dt.float32r
BF16 = mybir.dt.bfloat16
F8 = mybir.dt.float8e4
I16 = mybir.dt.int16
DR = mybir.MatmulPerfMode.DoubleRow
F8NP = ml_dtypes.float8_e4m3

SW = 128.0        # w1/w2 quant scale
SW3 = 16.0        # shared w3 quant scale (folds the 1/8 shared combine)
SG = 32.0         # gate scale (folds the 1/4 routed combine at h scale 128)
SOUT = 1.0 / (SW * SW)
BIG = 65536.0


def _legalize_waits(nc):
    """Split multi-wait instructions into single-wait NOP prefixes (the
    walrus pass list used by the bass2jax compile path has no sync
    legalization pass; cayman 64B instructions carry one wait slot)."""
    n_split = 0
    for fn in nc.m.functions:
        for blk in fn.blocks:
            out = []
            changed = False
            for inst in blk.instructions:
                si = inst.sync_info
                waits = list(si.on_wait) if si is not None and si.on_wait else []
                if len(waits) > 1:
                    for w in waits[:-1]:
                        nop = mybir.InstNoOp(
                            name=nc.get_next_instruction_name(),
                            engine=inst.engine,
                            bass_nofuse=True,
                            sync_info=mybir.SyncInfo(on_wait=[w], on_update=[]),
                        )
                        nc.register_instruction(nop)
                        out.append(nop)
                    si.on_wait = [waits[-1]]
                    inst.sync_info = si
                    n_split += 1
                    changed = True
                out.append(inst)
            if changed:
                blk.instructions = out
    return n_split


def _build_nc(caps):
    """caps: tuple of E per-expert capacities (multiples of 128, may be 0)."""
    caps = tuple(int(c) for c in caps)
    assert all(0 <= c <= CAPMAX and c % P == 0 for c in caps), caps
    nc = bass.Bass()

    xT = nc.declare_dram_parameter("xT", [D, TOK], F32, isOutput=False)
    x8h_d = nc.declare_dram_parameter("x8h", [D, TOK], F8, isOutput=False)
    x8l_d = nc.declare_dram_parameter("x8l", [D, TOK], F8, isOutput=False)
    xpr_d = nc.declare_dram_parameter("xpr", [TOK + 16, 2 * D], F8, isOutput=False)
    tlog_d = nc.declare_dram_parameter("tlog", [TOK, E], F32, isOutput=False)
    wrx = nc.declare_dram_parameter("wrx", [D, E], F32, isOutput=False)
    ident = nc.declare_dram_parameter("ident", [P, P], F32, isOutput=False)
    iota_e = nc.declare_dram_parameter("iota", [P, E], F32, isOutput=False)
    u128 = nc.declare_dram_parameter("u128", [P, P], F32, isOutput=False)
    u8x = nc.declare_dram_parameter("u8x", [8, 8], F32, isOutput=False)
    tilesel = nc.declare_dram_parameter("tilesel", [P, 64], F32, isOutput=False)
    rowsel = nc.declare_dram_parameter("rowsel", [8, 1024], F32, isOutput=False)
    iotat = nc.declare_dram_parameter("iotat", [P, 64], F32R, isOutput=False)
    iotac = nc.declare_dram_parameter("iotac", [P, 4 * CAPMAX], F32, isOutput=False)
    w13s = nc.declare_dram_parameter("w13s", [P, NS * MS * 4 * NKD * P], F8, isOutput=False)
    w1r = nc.declare_dram_parameter("w1r", [P, E * MR * 2 * NKD * P], F8, isOutput=False)
    w2sp = nc.declare_dram_parameter("w2sp", [P, NKD * W2SEG], F8, isOutput=False)
    w2rt = nc.declare_dram_parameter("w2rt", [P, E * 2 * MR * D], F8, isOutput=False)
    yout = nc.declare_dram_parameter("yout", [TOK, D], F32, isOutput=True)

    AF = mybir.ActivationFunctionType
    ALU = mybir.AluOpType
    AX = mybir.AxisListType

    from contextlib import ExitStack
    with TileContext(nc) as tc:
        with ExitStack() as stk:
            def pool(name, bufs, space=None):
                kw = dict(space=space) if space else {}
                return stk.enter_context(tc.tile_pool(name=name, bufs=bufs, **kw))

            xpool = pool("xpool", 1)
            xfpool = pool("xfpool", 2)
            cpool = pool("cpool", 1)
            hpool = pool("hpool", 1)          # shared H (big)
            yapool = pool("yapool", 1)        # shared y accumulator
            w13pool = pool("w13pool", 2)
            w1rpool = pool("w1rpool", 4)
            w2spool = pool("w2spool", 2)
            w2rpool = pool("w2rpool", 2)
            rpool = pool("rpool", 1)          # router persistent tiles
            spool = pool("spool", 2)          # scratch vector tiles
            Spool = pool("Spool", 2)          # one-hot S tiles
            gxpool = pool("gxpool", 2)        # gathered x pairs
            ghpool = pool("ghpool", 1)        # routed H tiles
            gbpool = pool("gbpool", 2)
            ixpool = pool("ixpool", 1)        # wrapped idx int16 (per-e tags)
            apool = pool("apool", 2)
            hfpool = pool("hfpool", 2)
            ypool = pool("ypool", 2)          # shared transpose staging
            ytpool = pool("ytpool", 1)        # routed scatter staging
            dpool = pool("dpool", 4, "DRAM")
            pp_h1 = pool("pp_h1", 2, "PSUM")
            pp_h3 = pool("pp_h3", 2, "PSUM")
            pp_y = pool("pp_y", 2, "PSUM")
            pp_s = pool("pp_s", 2, "PSUM")

            nc.gpsimd.load_library(library_config.mlp)

            # ---------------- consts ----------------
            id_t = cpool.tile([P, P], F32)
            nc.sync.dma_start(out=id_t[:], in_=ident[:, :])
            io_t = cpool.tile([P, E], F32)
            nc.sync.dma_start(out=io_t[:], in_=iota_e[:, :])
            u_t = cpool.tile([P, P], F32)
            nc.sync.dma_start(out=u_t[:], in_=u128[:, :])
            u8_t = cpool.tile([8, 8], F32)
            nc.sync.dma_start(out=u8_t[:], in_=u8x[:, :])
            ts_t = cpool.tile([P, 64], F32)
            nc.sync.dma_start(out=ts_t[:], in_=tilesel[:, :])
            rs_t = cpool.tile([8, 1024], F32)
            nc.sync.dma_start(out=rs_t[:], in_=rowsel[:, :])
            it_t = cpool.tile([P, 64], F32R)
            nc.sync.dma_start(out=it_t[:], in_=iotat[:, :])
            ic_t = cpool.tile([P, 4 * CAPMAX], F32)
            nc.sync.dma_start(out=ic_t[:], in_=iotac[:, :])
            wr_t = cpool.tile([P, NKD * E], F32)     # [p, kd, e]
            nc.sync.dma_start(
                out=wr_t[:].rearrange("p (kd c) -> p kd c", c=E),
                in_=wrx.rearrange("(kd p) c -> p kd c", p=P),
            )
            ones32 = cpool.tile([1, P], F32)
            nc.vector.memset(ones32[:], 1.0)
            ones_r = cpool.tile([1, P], F32R)
            nc.vector.tensor_copy(ones_r[:], ones32[:])

            x8h = xpool.tile([P, NKD * TOK], F8)     # [p, kd, tok]
            x8l = xpool.tile([P, NKD * TOK], F8)
            nc.sync.dma_start(
                out=x8h[:].rearrange("p (kd t) -> p kd t", t=TOK),
                in_=x8h_d.rearrange("(kd p) t -> p kd t", p=P),
            )
            nc.sync.dma_start(
                out=x8l[:].rearrange("p (kd t) -> p kd t", t=TOK),
                in_=x8l_d.rearrange("(kd p) t -> p kd t", p=P),
            )
            xhv = x8h[:].rearrange("p (kd t) -> p kd t", t=TOK)
            xlv = x8l[:].rearrange("p (kd t) -> p kd t", t=TOK)

            # ---------------- router + masks + prefix ----------------
            NT = TOK // P                          # 8 token tiles
            gates_all = rpool.tile([P, NT * E], F32R)
            cm_all = rpool.tile([P, NT * E], F32)
            pref_all = rpool.tile([P, NT * E], F32)
            rkm_all = rpool.tile([P, NT * E], F32)

            # Emission is interleaved: router tiles / prefix / extraction
            # are woven between shared-expert matmul blocks so the PE never
            # sits idle waiting on the DVE routing chains.
            tot_box = {}

            def emit_router_tile(tt):
                if tt == 0:
                    tot_t = pp_y.tile([P, NV], F32, space="PSUM", tag="py")
                    tot_box["ps"] = tot_t[0:8, 0:E]
                tot_ps = tot_box["ps"]
                xf = xfpool.tile([P, NKD * P], F32, tag="xf")
                nc.sync.dma_start(
                    out=xf[:].rearrange("p (kd t) -> p kd t", t=P),
                    in_=xT.rearrange("(kd p) t -> p kd t", p=P)[:, :, tt * P:(tt + 1) * P],
                )
                L_t = pp_h1.tile([P, NV], F32, space="PSUM", tag="ph1")
                L_ps = L_t[:, 0:E]
                for kd in range(NKD):
                    nc.tensor.matmul(
                        L_ps,
                        xf[:, kd * P:(kd + 1) * P],
                        wr_t[:, kd * E:(kd + 1) * E],
                        start=(kd == 0), stop=(kd == NKD - 1),
                    )
                tl_t = spool.tile([P, E], F32, tag="rt_tl")
                nc.sync.dma_start(out=tl_t[:], in_=tlog_d[tt * P:(tt + 1) * P, :])
                Lt = spool.tile([P, E], F32, tag="rt_L")
                nc.vector.tensor_copy(Lt[:], L_ps)
                Lx = spool.tile([P, E], F32, tag="rt_Lx")
                nc.vector.tensor_tensor(out=Lx[:], in0=Lt[:], in1=tl_t[:], op=ALU.add)
                St = spool.tile([P, E], F32, tag="rt_S")
                nc.scalar.activation(St[:], Lx[:], AF.Sigmoid)

                # top-1 (lowest index wins ties)
                m1 = spool.tile([P, 1], F32, tag="rt_m1")
                nc.vector.reduce_max(m1[:], Lx[:], axis=AX.X)
                eq1 = spool.tile([P, E], F32, tag="rt_eq1")
                nc.vector.tensor_scalar(eq1[:], Lx[:], m1[:, 0:1], None, op0=ALU.is_ge)
                pen1 = spool.tile([P, E], F32, tag="rt_pen1")
                nc.vector.tensor_scalar(pen1[:], eq1[:], -1e9, 1e9,
                                        op0=ALU.mult, op1=ALU.add)
                ix1 = spool.tile([P, E], F32, tag="rt_ix1")
                nc.vector.tensor_tensor(out=ix1[:], in0=io_t[:], in1=pen1[:], op=ALU.add)
                i1 = spool.tile([P, 1], F32, tag="rt_i1")
                nc.vector.tensor_reduce(i1[:], ix1[:], axis=AX.X, op=ALU.min)
                mask1 = spool.tile([P, E], F32, tag="rt_mask1")
                nc.vector.tensor_scalar(mask1[:], io_t[:], i1[:, 0:1], None,
                                        op0=ALU.is_equal)

                # top-2
                neg1 = spool.tile([P, E], F32, tag="rt_neg1")
                nc.vector.tensor_scalar(neg1[:], mask1[:], -1e30, None, op0=ALU.mult)
                L2 = spool.tile([P, E], F32, tag="rt_L2")
                nc.vector.tensor_tensor(out=L2[:], in0=Lx[:], in1=neg1[:], op=ALU.add)
                m2t = spool.tile([P, 1], F32, tag="rt_m2")
                nc.vector.reduce_max(m2t[:], L2[:], axis=AX.X)
                eq2 = spool.tile([P, E], F32, tag="rt_eq2")
                nc.vector.tensor_scalar(eq2[:], L2[:], m2t[:, 0:1], None, op0=ALU.is_ge)
                pen2 = spool.tile([P, E], F32, tag="rt_pen2")
                nc.vector.tensor_scalar(pen2[:], eq2[:], -1e9, 1e9,
                                        op0=ALU.mult, op1=ALU.add)
                ix2 = spool.tile([P, E], F32, tag="rt_ix2")
                nc.vector.tensor_tensor(out=ix2[:], in0=io_t[:], in1=pen2[:], op=ALU.add)
                i2 = spool.tile([P, 1], F32, tag="rt_i2")
                nc.vector.tensor_reduce(i2[:], ix2[:], axis=AX.X, op=ALU.min)
                mask2 = spool.tile([P, E], F32, tag="rt_mask2")
                nc.vector.tensor_scalar(mask2[:], io_t[:], i2[:, 0:1], None,
                                        op0=ALU.is_equal)

                mask = spool.tile([P, E], F32, tag="rt_mask")
                nc.vector.tensor_tensor(out=mask[:], in0=mask1[:], in1=mask2[:], op=ALU.add)
                sm = spool.tile([P, E], F32, tag="rt_sm")
                nc.vector.tensor_tensor(out=sm[:], in0=St[:], in1=mask[:], op=ALU.mult)
                den = spool.tile([P, 1], F32, tag="rt_den")
                nc.vector.reduce_sum(den[:], sm[:], axis=AX.X)
                den2 = spool.tile([P, 1], F32, tag="rt_den2")
                nc.vector.tensor_scalar(den2[:], den[:], 1e-9, None, op0=ALU.add)
                rec = spool.tile([P, 1], F32, tag="rt_rec")
                nc.vector.reciprocal(rec[:], den2[:])
                recs = spool.tile([P, 1], F32, tag="rt_recs")
                nc.vector.tensor_scalar(recs[:], rec[:], SG, None, op0=ALU.mult)
                esl = slice(tt * E, (tt + 1) * E)
                nc.vector.tensor_scalar(gates_all[:, esl], sm[:], recs[:, 0:1],
                                        None, op0=ALU.mult)
                nc.vector.tensor_copy(cm_all[:, esl], mask[:])

                # in-tile inclusive prefix over tokens (contraction over t')
                pf_t = pp_h1.tile([P, NV], F32, space="PSUM", tag="ph1")
                pf_ps = pf_t[:, 0:E]
                nc.tensor.matmul(pf_ps, u_t[:], mask[:], start=True, stop=True)
                nc.vector.tensor_copy(pref_all[:, esl], pf_ps)
                # tile totals accumulate into row tt of [8, E]
                nc.tensor.matmul(tot_ps, ts_t[:, tt * 8:(tt + 1) * 8], mask[:],
                                 start=(tt == 0), stop=(tt == NT - 1))

            def emit_prefix_ranks():
                tot_sb = spool.tile([8, E], F32, tag="tot")
                nc.vector.tensor_copy(tot_sb[:], tot_box["ps"])
                off_t = pp_y.tile([P, NV], F32, space="PSUM", tag="py")
                off_ps = off_t[0:8, 0:E]
                nc.tensor.matmul(off_ps, u8_t[:], tot_sb[:], start=True, stop=True)
                off_sb = spool.tile([8, E], F32, tag="off")
                nc.vector.tensor_copy(off_sb[:], off_ps)

                for tt in range(NT):
                    esl = slice(tt * E, (tt + 1) * E)
                    ob_t = pp_h1.tile([P, NV], F32, space="PSUM", tag="ph1")
                    ob_ps = ob_t[:, 0:E]
                    nc.tensor.matmul(ob_ps, rs_t[:, tt * P:(tt + 1) * P], off_sb[:],
                                     start=True, stop=True)
                    rk = spool.tile([P, E], F32, tag="rk")
                    nc.vector.tensor_copy(rk[:], ob_ps)
                    rk2 = spool.tile([P, E], F32, tag="rk2")
                    nc.vector.tensor_tensor(out=rk2[:], in0=rk[:],
                                            in1=pref_all[:, esl], op=ALU.add)
                    # rank = rk2 - 1 where member else -BIG:
                    rk3 = spool.tile([P, E], F32, tag="rk3")
                    nc.vector.tensor_scalar(rk3[:], rk2[:], BIG - 1.0, None,
                                            op0=ALU.add)
                    rk4 = spool.tile([P, E], F32, tag="rk4")
                    nc.vector.tensor_tensor(out=rk4[:], in0=rk3[:],
                                            in1=cm_all[:, esl], op=ALU.mult)
                    nc.vector.tensor_scalar(rkm_all[:, esl], rk4[:], -BIG, None,
                                            op0=ALU.add)

            # ---------------- per-expert token lists ----------------
            idx16 = {}
            elist = [e for e in range(E) if caps[e] > 0]
            NJX = max(caps) // P
            grow_all = rpool.tile([8, CAPMAX], F32R)

            def emit_extract(e):
                C = caps[e]
                ex_t = pp_y.tile([P, NV], F32, space="PSUM", tag="py")
                ex_ps = ex_t[0:9, 0:C]
                icoff = (C // P - 1) * CAPMAX
                for tt in range(NT):
                    esl = tt * E + e
                    stg = spool.tile([P, 9], F32R, tag="stg")
                    nc.vector.tensor_copy(stg[:, 0:8], it_t[:, tt * 8:(tt + 1) * 8])
                    nc.vector.tensor_copy(stg[:, 8:9], gates_all[:, esl:esl + 1])
                    S_t = Spool.tile([P, CAPMAX], F32R, tag="S")
                    nc.vector.tensor_scalar(S_t[:, :C], ic_t[:, icoff:icoff + C],
                                            rkm_all[:, esl:esl + 1], None,
                                            op0=ALU.is_equal)
                    nc.tensor.matmul(ex_ps, stg[:, 0:9], S_t[:, :C],
                                     start=(tt == 0), stop=(tt == NT - 1))
                gall = spool.tile([16, CAPMAX], F32, tag="gall")
                nc.vector.tensor_copy(gall[0:9, :C], ex_t[0:9, 0:C])
                # iotat carries (token_id - TOK): empty slots sum to 0, so
                # +TOK points them at the sink row / zero pad row.
                ixa = spool.tile([8, CAPMAX], F32, tag="ixa")
                nc.vector.tensor_scalar(ixa[:, :C], gall[0:8, :C], float(TOK),
                                        None, op0=ALU.add)
                ixi = spool.tile([8, CAPMAX], I16, tag="ixi")
                nc.vector.tensor_copy(ixi[:, :C], ixa[:, :C])
                nc.sync.dma_start(out=grow_all[e:e + 1, :C].bitcast(F32),
                                  in_=gall[8:9, :C])

                dsc = dpool.tile([8, C], I16, tag=f"dsc{e % 4}")
                nc.sync.dma_start(out=dsc[:, :], in_=ixi[:8, :C])
                ixw = ixpool.tile([P, C // 16], I16, tag=f"ix{e}")
                nc.sync.dma_start(
                    out=ixw[:],
                    in_=dsc[:, :].rearrange("r (g m) -> (r g) m", g=16),
                )
                idx16[e] = ixw

            # ---------------- shared experts (per-expert groups) ------------
            # H for ONE shared expert at a time (halves SBUF vs both).
            y_acc = yapool.tile([P, NKD * TOK], BF16)     # [p, m2, tok]

            def triple(ps, wh, wl, rh, rl, first, last, kpr):
                for kp in range(kpr):
                    ks = slice(2 * kp, 2 * kp + 2)
                    nc.tensor.matmul(ps, wh[:, ks], rh[:, ks],
                                     start=(first and kp == 0), stop=False,
                                     perf_mode=DR)
                    nc.tensor.matmul(ps, wh[:, ks], rl[:, ks],
                                     start=False, stop=False, perf_mode=DR)
                    nc.tensor.matmul(ps, wl[:, ks], rh[:, ks],
                                     start=False, stop=(last and kp == kpr - 1),
                                     perf_mode=DR)

            for se in range(NS):
                hh = hpool.tile([P, MS * TOK], F8, tag="hh")
                hl = hpool.tile([P, MS * TOK], F8, tag="hl")
                hhv = hh[:].rearrange("p (kh t) -> p kh t", t=TOK)
                hlv = hl[:].rearrange("p (kh t) -> p kh t", t=TOK)
                for m in range(MS):
                    if se == 0 and m < NT:
                        emit_router_tile(m)
                    w13 = w13pool.tile([P, 4 * NKD * P], F8, tag="w13")
                    off = (se * MS + m) * 4 * NKD * P
                    nc.sync.dma_start(out=w13[:], in_=w13s[:, off:off + 4 * NKD * P])
                    wv = w13[:].rearrange("p (v ks mm) -> p v ks mm", v=4, mm=P)
                    for n in range(NN):
                        nsl = slice(n * NV, (n + 1) * NV)
                        ps1 = pp_h1.tile([P, NV], F32, space="PSUM", tag="ph1")
                        triple(ps1[:], wv[:, 0], wv[:, 1],
                               xhv[:, :, nsl], xlv[:, :, nsl], True, True, KPD)
                        ps3 = pp_h3.tile([P, NV], F32, space="PSUM", tag="ph3")
                        triple(ps3[:], wv[:, 2], wv[:, 3],
                               xhv[:, :, nsl], xlv[:, :, nsl], True, True, KPD)
                        sil = apool.tile([P, NV], BF16, tag="sil")
                        nc.scalar.activation(sil[:], ps1[:], AF.Silu, scale=1.0 / SW)
                        hf = hfpool.tile([P, NV], BF16, tag="hf")
                        nc.vector.tensor_tensor(out=hf[:], in0=sil[:], in1=ps3[:],
                                                op=ALU.mult)
                        nc.scalar.copy(hhv[:, m, nsl], hf[:])
                        nc.vector.tensor_tensor(out=hlv[:, m, nsl], in0=hf[:],
                                                in1=hhv[:, m, nsl], op=ALU.subtract)
                    if se == 0 and m == NT - 1:
                        emit_prefix_ranks()

                # phase 2 for this shared expert: accumulate into y_acc
                for m2 in range(NKD):
                    w2t = w2spool.tile([P, W2SEG // 2], F8, tag="w2s")
                    off = m2 * W2SEG + se * (W2SEG // 2)
                    nc.sync.dma_start(out=w2t[:], in_=w2sp[:, off:off + W2SEG // 2])
                    w2v = w2t[:].rearrange("p (v ks mm) -> p v ks mm", v=2, mm=P)
                    for n in range(NN):
                        nsl = slice(n * NV, (n + 1) * NV)
                        py = pp_y.tile([P, NV], F32, space="PSUM", tag="py")
                        nmm = (MS // 2) * 3
                        i = 0
                        for kp in range(MS // 2):
                            kh = slice(2 * kp, 2 * kp + 2)
                            ks = slice(2 * kp, 2 * kp + 2)
                            nc.tensor.matmul(py[:], w2v[:, 0, ks, :],
                                             hhv[:, kh, nsl],
                                             start=(i == 0), stop=False,
                                             perf_mode=DR)
                            i += 1
                            nc.tensor.matmul(py[:], w2v[:, 0, ks, :],
                                             hlv[:, kh, nsl],
                                             start=False, stop=False,
                                             perf_mode=DR)
                            i += 1
                            nc.tensor.matmul(py[:], w2v[:, 1, ks, :],
                                             hhv[:, kh, nsl],
                                             start=False, stop=(i == nmm - 1),
                                             perf_mode=DR)
                            i += 1
                        ysl = y_acc[:, m2 * TOK + n * NV:m2 * TOK + n * NV + NV]
                        if se == 0:
                            nc.scalar.copy(ysl, py[:])
                        else:
                            nc.vector.tensor_tensor(out=ysl, in0=ysl, in1=py[:],
                                                    op=ALU.add)
                    if se == 0 and m2 < E and caps[m2] > 0:
                        emit_extract(m2)

            # ---------------- shared transpose -> y_dram rows ----------------
            id_bf = cpool.tile([P, P], BF16)
            nc.vector.tensor_copy(id_bf[:], id_t[:])
            y_dram = dpool.tile([TOK + P, D], F32, tag="ydram")
            for tt in range(NT):
                yts = ypool.tile([P, D], F32, tag="yts")
                for m2 in range(NKD):
                    tr_ps = pp_s.tile([P, P], BF16, space="PSUM", tag="ps_tr")
                    nc.tensor.transpose(
                        out=tr_ps[:],
                        in_=y_acc[:, m2 * TOK + tt * P:m2 * TOK + (tt + 1) * P],
                        identity=id_bf[:])
                    nc.scalar.mul(yts[:, m2 * P:(m2 + 1) * P], tr_ps[:], SOUT)
                nc.sync.dma_start(out=y_dram[tt * P:(tt + 1) * P, :], in_=yts[:])

            # ---------------- routed experts (sparse) ----------------
            for e in elist:
                C = caps[e]
                NJ = C // P
                g_t = gxpool.tile([P, 16 * C], F8, tag="xg")
                nc.gpsimd.dma_gather(
                    g_t[:].rearrange("p (a b) -> p a b", a=16),
                    xpr_d[:, :], idx16[e][:], C, C, 2 * D, transpose=True)
                xv = g_t[:].rearrange("p (c i v) -> p c i v", c=NKD, v=2)
                xgh = xv[:, :, :, 0]
                xgl = xv[:, :, :, 1]

                gb_ps = pp_h3.tile([P, CAPMAX], F32, space="PSUM", tag="ph3")
                gr0 = gbpool.tile([1, CAPMAX], F32R, tag="gr0")
                nc.sync.dma_start(out=gr0[0:1, :C], in_=grow_all[e:e + 1, :C])
                gr_v = (gr0[0:1, :C]
                        .rearrange("o (g m) -> o g m", g=16)
                        .rearrange("o g m -> o m g"))
                gb_v = gb_ps[:, :C].rearrange("p (m g) -> p m g", g=16)
                nc.tensor.matmul(gb_v, ones_r[0:1, :], gr_v,
                                 start=True, stop=True)
                gb = gbpool.tile([P, CAPMAX], F32, tag="gb")
                nc.vector.tensor_copy(gb[:, :C], gb_ps[:, :C])

                # phase 1: H = gelu(x W1) * gate   [h-part, C]
                ghh = ghpool.tile([P, MR * CAPMAX], F8, tag="ghh")
                ghl = ghpool.tile([P, MR * CAPMAX], F8, tag="ghl")
                ghhv = ghh[:].rearrange("p (kh c) -> p kh c", c=CAPMAX)
                ghlv = ghl[:].rearrange("p (kh c) -> p kh c", c=CAPMAX)
                for m in range(MR):
                    w1t = w1rpool.tile([P, 2 * NKD * P], F8, tag="w1r")
                    off = (e * MR + m) * 2 * NKD * P
                    nc.sync.dma_start(out=w1t[:], in_=w1r[:, off:off + 2 * NKD * P])
                    wv = w1t[:].rearrange("p (v ks mm) -> p v ks mm", v=2, mm=P)
                    ps1 = pp_h1.tile([P, CAPMAX], F32, space="PSUM", tag="ph1")
                    triple(ps1[:, :C], wv[:, 0], wv[:, 1],
                           xgh[:, :, :C], xgl[:, :, :C], True, True, KPD)
                    gel = apool.tile([P, CAPMAX], BF16, tag="gel")
                    nc.scalar.activation(gel[:, :C], ps1[:, :C], AF.Gelu,
                                         scale=1.0 / SW)
                    hf = hfpool.tile([P, CAPMAX], BF16, tag="ghf")
                    nc.vector.tensor_tensor(out=hf[:, :C], in0=gel[:, :C],
                                            in1=gb[:, :C], op=ALU.mult)
                    nc.scalar.copy(ghhv[:, m, :C], hf[:, :C])
                    nc.vector.tensor_tensor(out=ghlv[:, m, :C], in0=hf[:, :C],
                                            in1=ghhv[:, m, :C], op=ALU.subtract)

                # phase 2 flipped: Y^T [slot, d] ; contraction over h
                yt = ytpool.tile([P, NJX * D], F32, tag="yt")
                ytv = yt[:, :NJ * D].rearrange("p (j d) -> p j d", d=D)
                for half in range(2):
                    dsl = slice(half * (D // 2), (half + 1) * (D // 2))
                    pys = []
                    for j in range(NJ):
                        if j < 2:
                            pyj = pp_y.tile([P, D // 2], F32, space="PSUM",
                                            tag="py", name=f"pyj{j}")
                        else:
                            pyj = pp_h1.tile([P, D // 2], F32, space="PSUM",
                                             tag="ph1", name=f"pyj{j}")
                        pys.append(pyj)
                    for kp in range(MR // 2):
                        w2c = w2rpool.tile([P, 2 * 2 * (D // 2)], F8, tag="w2r")
                        cw = 2 * 2 * (D // 2)
                        coff = ((e * (MR // 2) + kp) * 2 + half) * cw
                        nc.sync.dma_start(out=w2c[:], in_=w2rt[:, coff:coff + cw])
                        wc = w2c[:].rearrange("p (v ks d) -> p v ks d", v=2, ks=2)
                        for j in range(NJ):
                            jsl = slice(j * P, (j + 1) * P)
                            hsl = slice(2 * kp, 2 * kp + 2)
                            first = (kp == 0)
                            last = (kp == MR // 2 - 1)
                            nc.tensor.matmul(pys[j][:], ghhv[:, hsl, jsl],
                                             wc[:, 0], start=first, stop=False,
                                             perf_mode=DR)
                            nc.tensor.matmul(pys[j][:], ghlv[:, hsl, jsl],
                                             wc[:, 0], start=False, stop=False,
                                             perf_mode=DR)
                            nc.tensor.matmul(pys[j][:], ghhv[:, hsl, jsl],
                                             wc[:, 1], start=False, stop=last,
                                             perf_mode=DR)
                    for j in range(NJ):
                        nc.scalar.mul(ytv[:, j, dsl], pys[j][:], SOUT)

                # scatter-add into y rows (exact fp32); pads add zeros to row 0
                nc.gpsimd.dma_scatter_add(
                    y_dram[:, :], ytv[:, :, :], idx16[e][:], C, C, D)

            # ---------------- final copy ----------------
            nc.sync.dma_start(out=yout[:, :], in_=y_dram[0:TOK, :])

    _legalize_waits(nc)
    mybir.codegen_inst_isa_subclasses(nc)
    return nc


# ---------------------------------------------------------------------------
_CACHE = {}


def _hilo(w, scale):
    v = np.asarray(w, np.float32) * scale
    hi = np.clip(v, -240.0, 240.0).astype(F8NP)
    lo = (v - hi.astype(np.float32)).astype(F8NP)
    return hi, lo


def _pack_in(a):
    E_, H, Dd = a.shape
    M = H // P
    KS = Dd // P
    a = a.reshape(E_, M, P, KS, P).transpose(4, 0, 1, 3, 2)
    return np.ascontiguousarray(a.reshape(P, E_, M, KS * P))


def _pack_out(a):
    E_, Dd, H = a.shape
    M2 = Dd // P
    KS = H // P
    return a.reshape(E_, M2, P, KS, P).transpose(4, 0, 1, 3, 2)


def _prep_weights(W_router, router_bias, s_w1, s_w3, s_w2, r_w1, r_w2):
    key = tuple(id(a) for a in (W_router, router_bias, s_w1, s_w3, s_w2, r_w1, r_w2))
    hit = _CACHE.get("wkey")
    if hit is not None and hit[0] == key:
        return hit[1]
    assert np.all(np.asarray(router_bias) == 0.0), "kernel assumes zero router bias"
    c = np.ascontiguousarray
    f = np.float32

    w1h, w1l = _hilo(s_w1, SW)
    w3h, w3l = _hilo(s_w3, SW3)
    parts = [_pack_in(a) for a in (w1h, w1l, w3h, w3l)]
    w13 = np.stack(parts, axis=3)
    w13 = c(w13.reshape(P, -1))

    r1h, r1l = _hilo(r_w1, SW)
    w1rp = np.stack([_pack_in(r1h), _pack_in(r1l)], axis=3)
    w1rp = c(w1rp.reshape(P, -1))

    # shared w2 blocks per m2 (dense path): [p, e, hi|lo, ks, mm]
    s2h, s2l = _hilo(s_w2, SW)
    w2s = np.stack([_pack_out(s2h), _pack_out(s2l)], axis=2)  # [p, e, 2, m2, ks, mm]
    segs = [w2s[:, :, :, m2].reshape(P, -1) for m2 in range(NKD)]
    w2spk = c(np.concatenate(segs, axis=1))

    # routed w2 transposed for flipped phase 2: contraction over h.
    # r_w2: [E, D, HR] -> w2T [E, HR, D] -> [p=h%128, e, v, ks=h//128, d]
    r2h, r2l = _hilo(r_w2, SW)
    def packT(a):
        aT = np.ascontiguousarray(np.transpose(a, (0, 2, 1)))   # [E, HR, D]
        return aT.reshape(E, MR, P, D).transpose(2, 0, 1, 3)    # [p, e, ks, d]
    w2rt = np.stack([packT(r2h), packT(r2l)], axis=2)           # [p, e, v, ks, d]
    # -> [p, e, kp, half, v, ks2, d512] contiguous per (e, kp, half) chunk
    w2rt = w2rt.reshape(P, E, 2, MR // 2, 2, 2, D // 2)         # [p,e,v,kp,ks2,half,d]
    w2rt = w2rt.transpose(0, 1, 3, 5, 2, 4, 6)                  # [p,e,kp,half,v,ks2,d]
    w2rt = c(w2rt.reshape(P, -1))

    wrTf = c(np.asarray(W_router, f).T)             # [2D, E]
    iotac = np.full((P, 4 * CAPMAX), -2.0, f)
    for k in range(4):
        C = (k + 1) * P
        j = np.arange(C)
        vals = (j % (C // 16)) * 16 + j // (C // 16)
        iotac[:, k * CAPMAX:k * CAPMAX + C] = vals[None, :].astype(f)
    u128 = np.triu(np.ones((P, P), f))              # u[t', t] = t' <= t
    u8x = np.triu(np.ones((8, 8), f), 1)            # exclusive
    tilesel = np.zeros((P, 8, 8), f)
    for i in range(8):
        tilesel[:, i, i] = 1.0
    rowsel = np.zeros((8, 8, P), f)
    for i in range(8):
        rowsel[i, i, :] = 1.0
    iotat = np.zeros((P, 8, 8), f)
    for tt in range(8):
        iotat[:, tt, :] = (tt * P + np.arange(P, dtype=f) - TOK)[:, None]

    prep = dict(
        w13s=w13, w1r=w1rp, w2sp=w2spk, w2rt=w2rt,
        wrx=c(wrTf[:D, :]),
        wrt_t=c(wrTf[D:, :]),
        iota=c(np.broadcast_to(np.arange(E, dtype=f), (P, E))),
        ident=np.eye(P, dtype=f),
        u128=c(u128), u8x=c(u8x),
        tilesel=c(tilesel.reshape(P, 64)),
        rowsel=c(rowsel.reshape(8, 1024)),
        iotat=c(iotat.reshape(P, 64)),
        iotac=c(iotac),
    )
    _CACHE["wkey"] = (key, prep)
    return prep


def _plan_routing(x, t_emb, W_router, router_bias):
    """Host-side planning: per-core token permutation + static capacities.

    Only sizes/placement come from here; the device recomputes routing."""
    f = np.float32
    N = B * T
    xf = np.asarray(x, f).reshape(N, D)
    W = np.asarray(W_router, f)
    tlogB = np.asarray(t_emb, f) @ W[:, D:].T          # [B, E]
    tlogN = np.repeat(tlogB, T, axis=0)                # [N, E]
    logits = xf @ W[:, :D].T + tlogN
    s = 1.0 / (1.0 + np.exp(-logits))
    sel = s + np.asarray(router_bias, f)
    top1 = np.argmax(sel, axis=1)
    sel2 = sel.copy()
    sel2[np.arange(N), top1] = -np.inf
    top2 = np.argmax(sel2, axis=1)

    # greedy balancing: assign each token to the core with the lightest
    # combined load on its two experts.
    load = np.zeros((N_CORES, E), np.int32)
    fill = np.zeros(N_CORES, np.int32)
    core_of = np.empty(N, np.int32)
    order = np.argsort(top1 * E + top2, kind="stable")
    for t in order:
        e1, e2 = top1[t], top2[t]
        best, bestv = -1, None
        for cix in range(N_CORES):
            if fill[cix] >= TOK:
                continue
            v = (int(load[cix][e1]) + int(load[cix][e2]), int(fill[cix]))
            if bestv is None or v < bestv:
                best, bestv = cix, v
        core_of[t] = best
        fill[best] += 1
        load[best][e1] += 1
        load[best][e2] += 1

    perm = np.argsort(core_of, kind="stable")          # tokens grouped by core
    caps = []
    for e in range(E):
        worst = int(load[:, e].max())
        caps.append(min(int(math.ceil((worst + 64) / P)) * P, CAPMAX))
    return perm, tuple(caps), tlogN


def kernel(x, t_emb, W_router, router_bias, s_w1, s_w3, s_w2, r_w1, r_w2):
    x = np.asarray(x, np.float32)
    pw = _prep_weights(W_router, router_bias, s_w1, s_w3, s_w2, r_w1, r_w2)
    perm, caps, tlogN = _plan_routing(x, t_emb, W_router, router_bias)
    _CACHE["caps"] = caps

    if _CACHE.get("nc_caps") != caps:
        _CACHE["nc"] = _build_nc(caps)
        _CACHE["nc_caps"] = caps
    nc = _CACHE["nc"]

    N = B * T
    x_rows = x.reshape(N, D)[perm]                     # permuted token rows
    xT_full = np.ascontiguousarray(x_rows.T)           # [D, N]
    xh_full, xl_full = _hilo(xT_full, 1.0)
    xh_rows, xl_rows = _hilo(x_rows, 1.0)
    xpr_full = np.stack([xh_rows, xl_rows], axis=-1).reshape(N, 2 * D)
    tlog_perm = np.ascontiguousarray(tlogN[perm])

    in_maps = []
    for cix in range(N_CORES):
        sl = slice(cix * TOK, (cix + 1) * TOK)
        in_maps.append(dict(
            xT=np.ascontiguousarray(xT_full[:, sl]),
            x8h=np.ascontiguousarray(xh_full[:, sl]),
            x8l=np.ascontiguousarray(xl_full[:, sl]),
            xpr=np.ascontiguousarray(np.concatenate(
                [xpr_full[sl], np.zeros((16, 2 * D), F8NP)], axis=0)),
            tlog=np.ascontiguousarray(tlog_perm[sl]),
            wrx=pw["wrx"], ident=pw["ident"], iota=pw["iota"],
            u128=pw["u128"], u8x=pw["u8x"], tilesel=pw["tilesel"],
            rowsel=pw["rowsel"], iotat=pw["iotat"], iotac=pw["iotac"],
            w13s=pw["w13s"], w1r=pw["w1r"], w2sp=pw["w2sp"], w2rt=pw["w2rt"],
        ))

    res = run_bass_kernel_spmd(nc, in_maps, list(range(N_CORES)))

    out_perm = np.empty((N, D), dtype=np.float32)
    for cix in range(N_CORES):
        out_perm[cix * TOK:(cix + 1) * TOK] = res.results[cix]["yout"]
    out = np.empty((N, D), dtype=np.float32)
    out[perm] = out_perm
    return out.reshape(B, T, D)
